# revision 17
# baseline (speedup 1.0000x reference)
"""Trainium2 Bass kernel for the EnhancedEncoderLayer (dense MHA + low-rank
top-k sparse attention + FFN, two layernorms).

Sharding: 8 cores = (batch b in 0..3) x (query-half h in {0,1}). Each core
computes output rows [b, h*512:(h+1)*512, :]. K/V-side projections are
computed redundantly per batch pair (no cross-core communication).

The host permutes src[b].T columns so each core's own query tokens are
columns 0..511 (attention contracts over all keys, so key order is
irrelevant); this keeps the SPMD program identical across cores.

v2 structure:
- score-path projections (q, k, sparse Q/K) run in f32r from f32 x / f32
  weights (exp amplifies score errors; bf16 operands there cost ~10x
  output error). V / Vsp / out_proj / FFN operands are bf16.
- V / Vsp are produced token-major directly by matmul (lhsT = x chunks),
  with the bias added via a K=1 ones-row matmul - no PE transposes.
- sparse probabilities computed in BOTH orientations by matmul (psb [q,k]
  for the threshold search, pspT [k,q] for the spmm); the threshold mask
  is applied in place on pspT. psb lives in a never-freed pool so the
  (slow, fully-overlapped) bisection never gates SBUF reuse.
- top-k threshold bisection: 12 iterations of bf16 DVE counting,
  overlapped under the v/vsp/attention phases.
- dense score matmuls issue as adjacent row-group pairs (head A rows
  0:64, head B rows 64:128) sharing one 2-bank PSUM tile and ONE exp.
- engine placement: ACT = exps + evacs (per-partition bias/scale);
  DVE = bisect, k/q evacs, LN, fuse, ctx normalize, masks; GpSimd =
  partition broadcasts + small DMAs ONLY (its ALU is ~6x slower than
  spec); Sync = weight streaming.
"""
import sys
import os
import contextlib

for _p in ('/opt/trn_rl_repo',):
    if _p not in sys.path:
        sys.path.insert(0, _p)

import numpy as np
import concourse.bacc as bacc
import concourse.tile as tile
from concourse import mybir
from concourse.bass_utils import run_bass_kernel_spmd
from concourse.masks import make_identity

F32 = mybir.dt.float32
F32R = mybir.dt.float32r
BF16 = mybir.dt.bfloat16
U32 = mybir.dt.uint32
AF = mybir.ActivationFunctionType
OP = mybir.AluOpType

B, S, D, H, R, DFF = 4, 1024, 1024, 16, 64, 4096
DH = D // H          # 64
SQ = S // 2          # 512 own queries per core
KK = max(1, int(S * 0.2))   # 204
KC = D // 128        # 8 contraction chunks over D
FC = DFF // 128      # 32 chunks over DFF
NQT = SQ // 128      # 4 query tiles
NTOK = S // 128      # 8 token tiles
BISECT_ITERS = 12
INV_SQRT = 0.125     # 1/sqrt(DH) == 1/sqrt(R)

# bpack column layout ([128, NBC] f32)
CQ, CK = 0, 8            # q / k in_proj bias (8 cols each)
CB1 = 24                 # ff1 bias (32 cols)
CG1, CBE1 = 56, 64       # ln1 gamma / beta (8 cols each)
CQP, CKP = 72, 73        # sparse Q/K proj bias (64 rows used)
NBC = 74
# rows layout ([NRW, D] f32)
RW_BO, RW_B12, RW_G1, RW_G2, RW_BE2, RW_BV, RW_BVP = range(7)
NRW = 7

_cached = {}


def _build():
    nc = bacc.Bacc()

    def din(name, shape, dt=F32):
        return nc.declare_dram_parameter(name, list(shape), dt, isOutput=False)

    xT = din("xT", [D, S])              # f32: q/k/sparse projections
    xb = din("xb", [D, S], BF16)        # bf16: v/vsp projections
    x_own = din("x_own", [SQ, D])       # own rows, token-major, f32
    wqkT = din("wqkT", [D, 2 * D])      # f32 (q cols 0:D, k cols D:2D)
    wvT = din("wvT", [D, D], BF16)
    woT = din("woT", [D, D], BF16)
    vpT = din("vpT", [D, D], BF16)
    qkpT = din("qkpT", [D, 2 * R])      # f32
    f1T = din("f1T", [D, DFF], BF16)
    f2T = din("f2T", [DFF, D], BF16)
    bpack = din("bpack", [128 * NBC])
    rows = din("rows", [NRW, D])
    lam = din("lam", [1, 1])
    out = nc.declare_dram_parameter("out", [SQ, D], F32, isOutput=True)
    DBG = bool(os.environ.get("BASSK_DEBUG"))
    if DBG:
        dbg_dense = nc.declare_dram_parameter("dbg_dense", [SQ, D], F32,
                                              isOutput=True)
        dbg_sparse = nc.declare_dram_parameter("dbg_sparse", [SQ, D], F32,
                                               isOutput=True)
        dbg_misc = nc.declare_dram_parameter("dbg_misc", [128, 16], F32,
                                             isOutput=True)

    xT_r = xT.ap().bitcast(F32R).rearrange("(kc p) s -> p kc s", p=128)
    xb_r = xb.ap().rearrange("(kc p) s -> p kc s", p=128)
    wqkT_r = wqkT.ap().bitcast(F32R).rearrange("(kc p) f -> p kc f", p=128)
    wvT_r = wvT.ap().rearrange("(kc p) f -> p kc f", p=128)
    woT_r = woT.ap().rearrange("(kc p) f -> p kc f", p=128)
    vpT_r = vpT.ap().rearrange("(kc p) f -> p kc f", p=128)
    qkpT_r = qkpT.ap().bitcast(F32R).rearrange("(kc p) f -> p kc f", p=128)
    f1T_r = f1T.ap().rearrange("(kc p) f -> p kc f", p=128)
    f2T_r = f2T.ap().rearrange("(kc p) f -> p kc f", p=128)
    bpack_r = bpack.ap().rearrange("(c p) -> p c", p=128)

    with tile.TileContext(nc) as tc:
        est = contextlib.ExitStack()
        with est:
            # ---------------- constants + small loads ----------------
            consts = est.enter_context(tc.tile_pool(name="consts", bufs=1))

            ident_f = consts.tile([128, 128], F32, name="ident_f")
            make_identity(nc, ident_f)

            eps_t = consts.tile([128, 1], F32, name="eps_t")
            nc.vector.memset(eps_t, 1e-5)
            ones1 = consts.tile([128, 1], F32, name="ones1")
            nc.vector.memset(ones1, 1.0)
            ones_bf = consts.tile([1, 128], BF16, name="ones_bf")
            nc.vector.memset(ones_bf, 1.0)
            ones_b1 = consts.tile([128, 1], BF16, name="ones_b1")
            nc.vector.memset(ones_b1, 1.0)

            lam_t = consts.tile([1, 1], F32, name="lam_t")
            nc.gpsimd.dma_start(out=lam_t, in_=lam.ap())
            bp = consts.tile([128, NBC], F32, name="bp")
            nc.gpsimd.dma_start(out=bp, in_=bpack_r)
            rtmp_stack = contextlib.ExitStack()
            rtmp = rtmp_stack.enter_context(
                tc.tile_pool(name="rtmp", bufs=2, side="right"))

            def load_row(i):
                rt = rtmp.tile([1, D], F32, name="rrow", tag="rrow")
                nc.gpsimd.dma_start(out=rt, in_=rows.ap()[i:i + 1, :])
                return rt

            sg_t = consts.tile([1, 1], F32, name="sg_t")
            nc.scalar.activation(out=sg_t, in_=lam_t, func=AF.Sigmoid)
            sig_bc = consts.tile([128, 1], F32, name="sig_bc")
            nc.gpsimd.partition_broadcast(sig_bc, sg_t)
            oms_bc = consts.tile([128, 1], F32, name="oms_bc")
            nc.vector.tensor_sub(oms_bc, ones1, sig_bc)

            # bf16 bias rows for the K=1 bias matmuls (v / vsp)
            bvb0 = consts.tile([1, D], BF16, name="bvb0")
            nc.vector.tensor_copy(out=bvb0, in_=load_row(RW_BV))
            bvb1 = consts.tile([1, D], BF16, name="bvb1")
            nc.vector.tensor_copy(out=bvb1, in_=load_row(RW_BVP))

            # own-token residual xot = x_own + sig*bo
            bo_sig = consts.tile([128, D], F32, name="bo_sig")
            nc.gpsimd.partition_broadcast(bo_sig, load_row(RW_BO))
            nc.vector.tensor_scalar_mul(bo_sig, bo_sig, sig_bc)

            xot_pool = est.enter_context(tc.tile_pool(name="xot_pool",
                                                      bufs=1))
            xot = xot_pool.tile([128, NQT, D], F32, name="xot")
            for qt in range(NQT):
                nc.gpsimd.dma_start(
                    out=xot[:, qt, :],
                    in_=x_own.ap()[qt * 128:qt * 128 + 128, :])
            for qt in range(NQT):
                nc.vector.tensor_add(xot[:, qt, :], xot[:, qt, :], bo_sig)

            # x chunks: f32r for the score path, bf16 for v/vsp
            x_pool = est.enter_context(tc.tile_pool(name="x_pool", bufs=1))
            xTt = x_pool.tile([128, KC, S], F32R, name="xTt")
            for kc in range(KC):
                nc.sync.dma_start(out=xTt[:, kc, :], in_=xT_r[:, kc, :])
            xbp_stack = contextlib.ExitStack()
            xbp_pool = xbp_stack.enter_context(
                tc.tile_pool(name="xbp_pool", bufs=1, side="right"))
            xb_t = xbp_pool.tile([128, KC, S], BF16, name="xb_t")
            for kc in range(KC):
                nc.sync.dma_start(out=xb_t[:, kc, :], in_=xb_r[:, kc, :])

            # ---------- long-lived small tiles (incl. bisect) ----------
            bis = est.enter_context(tc.tile_pool(name="bis", bufs=1))
            lo = bis.tile([128, NQT], F32, name="lo")
            hi = bis.tile([128, NQT], F32, name="hi")
            mid = bis.tile([128, NQT], F32, name="mid")
            cnts = bis.tile([128, NQT], F32, name="cnts")
            pred = bis.tile([128, NQT], U32, name="pred")
            rs_sp = bis.tile([128, NQT], F32, name="rs_sp")
            rcp_sp = bis.tile([128, NQT], F32, name="rcp_sp")
            loT_row = bis.tile([1, SQ], F32, name="loT_row")
            thrT = bis.tile([128, SQ], F32, name="thrT")
            psb = bis.tile([128, NQT, S], BF16, name="psb")
            scrb = bis.tile([128, S], BF16, name="scrb")

            ctx_stack = contextlib.ExitStack()
            ctx_pool = ctx_stack.enter_context(
                tc.tile_pool(name="ctx_pool", bufs=1))
            ctxT = ctx_pool.tile([128, KC, SQ], BF16, name="ctxT")

            spv_stack = contextlib.ExitStack()
            spv_pool = spv_stack.enter_context(
                tc.tile_pool(name="spv_pool", bufs=1))
            pspT = spv_pool.tile([128, NTOK, SQ], BF16, name="pspT")
            Vsp = spv_pool.tile([128, NTOK, D], BF16, name="Vsp")

            # ============ phase 0: sparse projections + scores ============
            sp_stack = contextlib.ExitStack()
            sp_pool = sp_stack.enter_context(
                tc.tile_pool(name="sp_pool", bufs=1))
            qkp_t = sp_pool.tile([128, KC, 2 * R], F32R, name="qkp_t")
            nc.gpsimd.dma_start(out=qkp_t, in_=qkpT_r)
            kspT_b = sp_pool.tile([64, S], BF16, name="kspT_b")
            qspT_b = sp_pool.tile([64, SQ], BF16, name="qspT_b")

            with contextlib.ExitStack() as ph0:
                ps_sp = ph0.enter_context(
                    tc.tile_pool(name="ps_sp", bufs=2, space="PSUM"))
                with nc.named_scope("p0_sparse"):
                    for nh in range(2):
                        ps = ps_sp.tile([64, 512], F32, name="ps0", tag="ps0")
                        for kc in range(KC):
                            nc.tensor.matmul(
                                ps, qkp_t[:, kc, R:2 * R],
                                xTt[:, kc, nh * 512:nh * 512 + 512],
                                start=(kc == 0), stop=(kc == KC - 1))
                        nc.scalar.activation(
                            out=kspT_b[:, nh * 512:nh * 512 + 512], in_=ps,
                            func=AF.Identity, bias=bp[0:64, CKP:CKP + 1],
                            scale=1.0)
                    ps = ps_sp.tile([64, 512], F32, name="ps0", tag="ps0")
                    for kc in range(KC):
                        nc.tensor.matmul(ps, qkp_t[:, kc, 0:R],
                                         xTt[:, kc, 0:SQ],
                                         start=(kc == 0), stop=(kc == KC - 1))
                    nc.scalar.activation(out=qspT_b, in_=ps,
                                         func=AF.Identity,
                                         bias=bp[0:64, CQP:CQP + 1],
                                         scale=1.0)

                with nc.named_scope("p1_ssp"):
                    # psb [q, k] for the threshold search (one exp per qt)
                    for qt in range(NQT):
                        ps = ps_sp.tile([128, 1024], F32, name="ps1",
                                        tag="ps1")
                        for nh in range(2):
                            nc.tensor.matmul(
                                ps[:, nh * 512:nh * 512 + 512],
                                qspT_b[:, qt * 128:qt * 128 + 128],
                                kspT_b[:, nh * 512:nh * 512 + 512],
                                start=True, stop=True)
                        nc.scalar.activation(
                            out=psb[:, qt, :], in_=ps, func=AF.Exp,
                            scale=INV_SQRT)
                    # pspT [k, q] for the spmm (one exp per kt pair)
                    for kt2 in range(NTOK // 2):
                        ps = ps_sp.tile([128, 1024], F32, name="ps1",
                                        tag="ps1")
                        for j in range(2):
                            kt = kt2 * 2 + j
                            nc.tensor.matmul(
                                ps[:, j * 512:j * 512 + 512],
                                kspT_b[:, kt * 128:kt * 128 + 128],
                                qspT_b, start=True, stop=True)
                        nc.scalar.activation(
                            out=pspT[:, kt2 * 2:kt2 * 2 + 2, :], in_=ps,
                            func=AF.Exp, scale=INV_SQRT)

                # ---- top-k threshold bisection (DVE, bf16 counting) ----
                # ~6.5us/iter; fully overlapped under v/vsp/attention
                with nc.named_scope("p2_bisect"):
                    nc.vector.memset(lo, 0.0)
                    nc.vector.memset(hi, 16.0)
                    for it in range(BISECT_ITERS):
                        nc.vector.tensor_tensor(mid, lo, hi, OP.add)
                        nc.vector.tensor_scalar_mul(mid, mid, 0.5)
                        for qt in range(NQT):
                            nc.vector.scalar_tensor_tensor(
                                out=scrb, in0=psb[:, qt, :],
                                scalar=mid[:, qt:qt + 1],
                                in1=ones_b1.to_broadcast([128, S]),
                                op0=OP.is_ge, op1=OP.mult,
                                accum_out=cnts[:, qt:qt + 1])
                        nc.vector.tensor_scalar(out=pred, in0=cnts,
                                                scalar1=float(KK),
                                                scalar2=None, op0=OP.is_ge)
                        nc.vector.copy_predicated(lo, pred, mid)
                        nc.vector.tensor_scalar(out=pred, in0=cnts,
                                                scalar1=float(KK),
                                                scalar2=None, op0=OP.is_lt)
                        nc.vector.copy_predicated(hi, pred, mid)
                    # row-sum of kept values (masked in place, [q,k] side)
                    for qt in range(NQT):
                        nc.vector.scalar_tensor_tensor(
                            out=psb[:, qt, :], in0=psb[:, qt, :],
                            scalar=lo[:, qt:qt + 1], in1=psb[:, qt, :],
                            op0=OP.is_ge, op1=OP.mult,
                            accum_out=rs_sp[:, qt:qt + 1])
                    nc.vector.tensor_scalar(out=rs_sp, in0=rs_sp,
                                            scalar1=1e-9, scalar2=None,
                                            op0=OP.add)
                    nc.vector.reciprocal(rcp_sp, rs_sp)
                    nc.vector.tensor_scalar_mul(rcp_sp, rcp_sp, oms_bc)
                    if DBG:
                        nc.sync.dma_start(out=dbg_misc.ap()[:, 0:4], in_=lo)
                        nc.sync.dma_start(out=dbg_misc.ap()[:, 4:8],
                                          in_=rs_sp)
                        nc.sync.dma_start(out=dbg_misc.ap()[:, 8:12],
                                          in_=rcp_sp)
                        nc.sync.dma_start(out=dbg_misc.ap()[:, 12:16],
                                          in_=cnts)
            sp_stack.close()

            dn_stack = contextlib.ExitStack()
            dn_pool = dn_stack.enter_context(
                tc.tile_pool(name="dn_pool", bufs=1))
            kT = dn_pool.tile([128, KC, S], BF16, name="kT")
            qT = dn_pool.tile([128, KC, SQ], BF16, name="qT")
            Vaug = dn_pool.tile([128, NTOK, H * (DH + 1)], BF16, name="Vaug")
            Vaug_h = Vaug.rearrange("p t (h c) -> p t h c", c=DH + 1)
            nc.gpsimd.memset(Vaug_h[:, :, :, DH:DH + 1], 1.0)

            # ============ phase 3/4: V and Vsp, token-major (bf16) ========
            with contextlib.ExitStack() as ph34:
                wv_pool = ph34.enter_context(
                    tc.tile_pool(name="wv_pool", bufs=3))
                ps_v = ph34.enter_context(
                    tc.tile_pool(name="ps_v", bufs=2, space="PSUM"))

                def vproj(w_r, brow, dest, scope):
                    with nc.named_scope(scope):
                        for nh in range(2):
                            wv = wv_pool.tile([128, KC, 512], BF16,
                                              name="wv", tag="wv")
                            nc.sync.dma_start(
                                out=wv,
                                in_=w_r[:, :, nh * 512:nh * 512 + 512])
                            for t in range(NTOK):
                                ps = ps_v.tile([128, 512], F32, name="psv",
                                               tag="psv")
                                for kc in range(KC):
                                    nc.tensor.matmul(
                                        ps, xb_t[:, kc, t * 128:t * 128 + 128],
                                        wv[:, kc, :],
                                        start=(kc == 0), stop=False)
                                nc.tensor.matmul(
                                    ps, ones_bf,
                                    brow[:, nh * 512:nh * 512 + 512],
                                    start=False, stop=True)
                                if dest is None:
                                    nc.scalar.activation(
                                        out=Vaug_h[:, t, nh * 8:nh * 8 + 8,
                                                   0:DH],
                                        in_=ps, func=AF.Copy, scale=1.0)
                                else:
                                    nc.scalar.activation(
                                        out=dest[:, t,
                                                 nh * 512:nh * 512 + 512],
                                        in_=ps, func=AF.Copy, scale=1.0)

                vproj(wvT_r, bvb0, None, "p3_v")
                vproj(vpT_r, bvb1, Vsp, "p4_vsp")
            xbp_stack.close()

            # ============ phase 5: k/q projections + dense attention ======
            with contextlib.ExitStack() as ph5:
                wstr = ph5.enter_context(tc.tile_pool(name="wstr", bufs=2))
                pt_pool = ph5.enter_context(
                    tc.tile_pool(name="pt_pool", bufs=6))
                rc_pool = ph5.enter_context(
                    tc.tile_pool(name="rc_pool", bufs=2))
                ps_proj = ph5.enter_context(
                    tc.tile_pool(name="ps_proj", bufs=2, space="PSUM"))
                ps_attn = ph5.enter_context(
                    tc.tile_pool(name="ps_attn", bufs=2, space="PSUM"))
                ps_ctx = ph5.enter_context(
                    tc.tile_pool(name="ps_ctx", bufs=2, space="PSUM"))

                with nc.named_scope("p5_kq_attn"):
                    for jj in range(4):
                        for fi in range(2):
                            ft = jj * 2 + fi
                            wkq = wstr.tile([128, KC, 256], F32R, name="wkq",
                                            tag="wkq")
                            nc.sync.dma_start(
                                out=wkq[:, :, 0:128],
                                in_=wqkT_r[:, :, D + ft * 128:
                                           D + ft * 128 + 128])
                            nc.sync.dma_start(
                                out=wkq[:, :, 128:256],
                                in_=wqkT_r[:, :, ft * 128:ft * 128 + 128])
                            for nh in range(2):
                                ps = ps_proj.tile([128, 512], F32, name="psp",
                                                  tag="psp")
                                for kc in range(KC):
                                    nc.tensor.matmul(
                                        ps, wkq[:, kc, 0:128],
                                        xTt[:, kc, nh * 512:nh * 512 + 512],
                                        start=(kc == 0), stop=(kc == KC - 1))
                                nc.vector.tensor_scalar(
                                    out=kT[:, ft, nh * 512:nh * 512 + 512],
                                    in0=ps, scalar1=bp[:, CK + ft:CK + ft + 1],
                                    scalar2=None, op0=OP.add)
                            ps = ps_proj.tile([128, 512], F32, name="psp",
                                              tag="psp")
                            for kc in range(KC):
                                nc.tensor.matmul(
                                    ps, wkq[:, kc, 128:256],
                                    xTt[:, kc, 0:SQ],
                                    start=(kc == 0), stop=(kc == KC - 1))
                            nc.vector.tensor_scalar(
                                out=qT[:, ft, :], in0=ps,
                                scalar1=bp[:, CQ + ft:CQ + ft + 1],
                                scalar2=None, op0=OP.add)
                        # attention for the 4 heads of these two f-tiles
                        for fi in range(2):
                            ft = jj * 2 + fi
                            pts = []
                            for t in range(NTOK):
                                ps = ps_attn.tile([128, 1024], F32,
                                                  name="ps_s", tag="ps_s")
                                for po in (0, 64):
                                    nc.tensor.matmul(
                                        ps[:, (po // 64) * 512:
                                           (po // 64) * 512 + 512],
                                        kT[po:po + 64, ft,
                                           t * 128:t * 128 + 128],
                                        qT[po:po + 64, ft, :],
                                        start=True, stop=True)
                                pt = pt_pool.tile([128, 1024], BF16,
                                                  name="pT", tag="pT")
                                nc.scalar.activation(out=pt, in_=ps,
                                                     func=AF.Exp,
                                                     scale=INV_SQRT)
                                pts.append(pt)
                            for po in (0, 64):
                                hh = 2 * ft + (po // 64)
                                qoff = (po // 64) * 512
                                pctx = ps_ctx.tile([128, 512], F32,
                                                   name="ps_c", tag="ps_c")
                                for t in range(NTOK):
                                    nc.tensor.matmul(
                                        pctx[0:65, :],
                                        Vaug[:, t, hh * 65:hh * 65 + 65],
                                        pts[t][:, qoff:qoff + 512],
                                        start=(t == 0),
                                        stop=(t == NTOK - 1))
                                rsr = rc_pool.tile([1, 512], F32, name="rsr",
                                                   tag="rsr")
                                nc.vector.tensor_copy(out=rsr,
                                                      in_=pctx[64:65, :])
                                rch = rc_pool.tile([1, 512], F32, name="rch",
                                                   tag="rch")
                                nc.vector.reciprocal_approx_fast(out=rch,
                                                                 in_=rsr)
                                rb = rc_pool.tile([64, 512], F32, name="rb",
                                                  tag="rb")
                                nc.gpsimd.partition_broadcast(rb, rch)
                                nc.vector.tensor_mul(
                                    out=ctxT[po:po + 64, ft, :],
                                    in0=pctx[0:64, :], in1=rb)

            dn_stack.close()   # free kT, qT, Vaug

            ds_stack = contextlib.ExitStack()
            ds_pool = ds_stack.enter_context(
                tc.tile_pool(name="ds_pool", bufs=1, side="right"))
            dense_s = ds_pool.tile([128, NQT, D], F32, name="dense_s")
            sparse_s = ds_pool.tile([128, NQT, D], F32, name="sparse_s")

            # ==== threshold row -> mask (overlaps out_proj below) ====
            with contextlib.ExitStack() as ph6a:
                ps_lo = ph6a.enter_context(
                    tc.tile_pool(name="ps_lo", bufs=2, space="PSUM"))
                with nc.named_scope("p6_loT"):
                    for qt in range(NQT):
                        pst = ps_lo.tile([1, 128], F32, name="pslo",
                                         tag="pslo")
                        nc.tensor.transpose(pst, lo[:, qt:qt + 1], ident_f)
                        nc.vector.tensor_copy(
                            out=loT_row[0:1, qt * 128:qt * 128 + 128],
                            in_=pst)
            with nc.named_scope("p6_mask"):
                nc.gpsimd.partition_broadcast(thrT, loT_row)
                for kt in range(NTOK):
                    nc.vector.tensor_tensor(scrb[:, 0:512], pspT[:, kt, :],
                                            thrT, OP.is_ge)
                    nc.vector.tensor_tensor(pspT[:, kt, :], pspT[:, kt, :],
                                            scrb[:, 0:512], OP.mult)

            # ============ phase 7: out_proj, then spmm ============
            with contextlib.ExitStack() as ph7:
                w2str = ph7.enter_context(tc.tile_pool(name="w2str", bufs=1))
                ps_mm = ph7.enter_context(
                    tc.tile_pool(name="ps_mm", bufs=2, space="PSUM"))
                with nc.named_scope("p7_outproj"):
                    wo2 = w2str.tile([128, KC, D], BF16, name="wo2",
                                     tag="wo2")
                    nc.sync.dma_start(out=wo2, in_=woT_r)
                    for qt in range(NQT):
                        ps = ps_mm.tile([128, 1024], F32, name="ps_o",
                                        tag="ps_o")
                        for kc in range(KC):
                            for nh in range(2):
                                nc.tensor.matmul(
                                    ps[:, nh * 512:nh * 512 + 512],
                                    ctxT[:, kc, qt * 128:qt * 128 + 128],
                                    wo2[:, kc, nh * 512:nh * 512 + 512],
                                    start=(kc == 0), stop=(kc == KC - 1))
                        nc.scalar.activation(
                            out=dense_s[:, qt, :], in_=ps, func=AF.Copy,
                            scale=sig_bc)
                with nc.named_scope("p8_spmm"):
                    for qt in range(NQT):
                        ps = ps_mm.tile([128, 1024], F32, name="ps_o",
                                        tag="ps_o")
                        for kt in range(NTOK):
                            for nh in range(2):
                                nc.tensor.matmul(
                                    ps[:, nh * 512:nh * 512 + 512],
                                    pspT[:, kt, qt * 128:qt * 128 + 128],
                                    Vsp[:, kt, nh * 512:nh * 512 + 512],
                                    start=(kt == 0), stop=(kt == NTOK - 1))
                        nc.scalar.activation(
                            out=sparse_s[:, qt, :], in_=ps, func=AF.Copy,
                            scale=rcp_sp[:, qt:qt + 1])
            if DBG:
                for qt in range(NQT):
                    nc.sync.dma_start(
                        out=dbg_dense.ap()[qt * 128:qt * 128 + 128, :],
                        in_=dense_s[:, qt, :])
                    nc.sync.dma_start(
                        out=dbg_sparse.ap()[qt * 128:qt * 128 + 128, :],
                        in_=sparse_s[:, qt, :])
            spv_stack.close()
            ctx_stack.close()

            # ============ phase 9: fuse + LN1 ============
            fse = est.enter_context(tc.tile_pool(name="fse", bufs=1))
            x78_stack = contextlib.ExitStack()
            x78 = x78_stack.enter_context(tc.tile_pool(name="x78", bufs=1))

            g1_bc = fse.tile([128, D], F32, name="g1_bc")
            b12_bc = fse.tile([128, D], F32, name="b12_bc")
            g2_bc = fse.tile([128, D], F32, name="g2_bc")
            be2_bc = fse.tile([128, D], F32, name="be2_bc")
            nc.gpsimd.partition_broadcast(g1_bc, load_row(RW_G1))
            nc.gpsimd.partition_broadcast(b12_bc, load_row(RW_B12))
            nc.gpsimd.partition_broadcast(g2_bc, load_row(RW_G2))
            nc.gpsimd.partition_broadcast(be2_bc, load_row(RW_BE2))

            xhat = x78.tile([128, NQT, D], F32, name="xhat")
            xg = fse.tile([128, NQT, D], F32, name="xg")
            stats = fse.tile([128, NQT, 2, 6], F32, name="stats")
            mv2 = fse.tile([128, NQT, 2], F32, name="mv2")
            sd = fse.tile([128, NQT], F32, name="sd")
            rstd = fse.tile([128, NQT], F32, name="rstd")

            def ln_normalize(x1, qt):
                for half in range(2):
                    nc.vector.bn_stats(
                        out=stats[:, qt, half, :],
                        in_=x1[:, half * 512:half * 512 + 512])
                nc.vector.bn_aggr(out=mv2[:, qt, :], in_=stats[:, qt])
                nc.scalar.activation(out=sd[:, qt:qt + 1],
                                     in_=mv2[:, qt, 1:2], func=AF.Sqrt,
                                     bias=eps_t, scale=1.0)
                nc.vector.reciprocal(rstd[:, qt:qt + 1], sd[:, qt:qt + 1])
                nc.vector.tensor_scalar(out=x1, in0=x1,
                                        scalar1=mv2[:, qt, 0:1],
                                        scalar2=rstd[:, qt:qt + 1],
                                        op0=OP.subtract, op1=OP.mult)

            with nc.named_scope("p9_fuse_ln1"):
                for qt in range(NQT):
                    x1 = xhat[:, qt, :]
                    nc.vector.tensor_add(x1, dense_s[:, qt, :],
                                         sparse_s[:, qt, :])
                    nc.vector.tensor_add(x1, x1, xot[:, qt, :])
                    ln_normalize(x1, qt)
                for qt in range(NQT):
                    nc.vector.tensor_mul(xg[:, qt, :], xhat[:, qt, :],
                                         g1_bc)
                    nc.vector.tensor_add(xg[:, qt, :], xg[:, qt, :],
                                         b12_bc)
            ds_stack.close()

            # ============ phase 10: xhat transpose -> ff1 input ============
            xln_stack = contextlib.ExitStack()
            xln_pool = xln_stack.enter_context(
                tc.tile_pool(name="xln_pool", bufs=1, side="right"))
            xlnT = xln_pool.tile([128, KC, SQ], BF16, name="xlnT")
            with contextlib.ExitStack() as ph10:
                ps_tr3 = ph10.enter_context(
                    tc.tile_pool(name="ps_tr3", bufs=2, space="PSUM"))
                with nc.named_scope("p10_xT"):
                    for qt in range(NQT):
                        for fc in range(KC):
                            pst = ps_tr3.tile([128, 128], F32, name="pst3",
                                              tag="pst3")
                            nc.tensor.transpose(
                                pst, xhat[:, qt, fc * 128:fc * 128 + 128],
                                ident_f)
                            nc.scalar.activation(
                                out=xlnT[:, fc, qt * 128:qt * 128 + 128],
                                in_=pst, func=AF.Identity,
                                bias=bp[:, CBE1 + fc:CBE1 + fc + 1],
                                scale=bp[:, CG1 + fc:CG1 + fc + 1])
            x78_stack.close()

            # ============ phase 11: ff1 + relu ============
            h1_stack = contextlib.ExitStack()
            h1_pool = h1_stack.enter_context(
                tc.tile_pool(name="h1_pool", bufs=1))
            h1T = h1_pool.tile([128, FC, SQ], BF16, name="h1T")
            with contextlib.ExitStack() as ph11:
                w3str = ph11.enter_context(tc.tile_pool(name="w3str", bufs=2))
                ps_f1 = ph11.enter_context(
                    tc.tile_pool(name="ps_f1", bufs=4, space="PSUM"))
                with nc.named_scope("p11_ff1"):
                    for jj in range(16):
                        wt = w3str.tile([128, KC, 256], BF16, name="w1t",
                                        tag="w3")
                        f0 = jj * 256
                        nc.sync.dma_start(out=wt,
                                          in_=f1T_r[:, :, f0:f0 + 256])
                        for fi in range(2):
                            dft = jj * 2 + fi
                            ps = ps_f1.tile([128, 512], F32, name="ps_f",
                                            tag="ps_f")
                            for kc in range(KC):
                                nc.tensor.matmul(
                                    ps, wt[:, kc, fi * 128:fi * 128 + 128],
                                    xlnT[:, kc, :],
                                    start=(kc == 0), stop=(kc == KC - 1))
                            nc.scalar.activation(
                                out=h1T[:, dft, :], in_=ps, func=AF.Relu,
                                bias=bp[:, CB1 + dft:CB1 + dft + 1],
                                scale=1.0)
            xln_stack.close()

            # ============ phase 12: ff2 + residual + LN2 + out ============
            ff_s = fse.tile([128, NQT, D], F32, name="ff_s")
            with contextlib.ExitStack() as ph12:
                w4str = ph12.enter_context(tc.tile_pool(name="w4str", bufs=2))
                ps_f2 = ph12.enter_context(
                    tc.tile_pool(name="ps_f2", bufs=4, space="PSUM"))
                with nc.named_scope("p12_ff2"):
                    pss = [ps_f2.tile([128, 1024], F32, name="ps_g",
                                      tag="ps_g") for _ in range(NQT)]
                    for k4 in range(FC // 4):
                        f2h = w4str.tile([128, 4, D], BF16, name="f2h",
                                         tag="w4")
                        nc.sync.dma_start(out=f2h,
                                          in_=f2T_r[:, k4 * 4:k4 * 4 + 4, :])
                        for j in range(4):
                            kc = k4 * 4 + j
                            for qt in range(NQT):
                                for nh in range(2):
                                    nc.tensor.matmul(
                                        pss[qt][:, nh * 512:nh * 512 + 512],
                                        h1T[:, kc, qt * 128:qt * 128 + 128],
                                        f2h[:, j, nh * 512:nh * 512 + 512],
                                        start=(kc == 0),
                                        stop=(kc == FC - 1))
                    for qt in range(NQT):
                        nc.scalar.activation(
                            out=ff_s[:, qt, :], in_=pss[qt], func=AF.Copy,
                            scale=1.0)

                with nc.named_scope("p12_ln2"):
                    for qt in range(NQT):
                        x2 = ff_s[:, qt, :]
                        nc.vector.tensor_add(x2, x2, xg[:, qt, :])
                        ln_normalize(x2, qt)
                        ot = fse.tile([128, D], F32, name="out_t",
                                      tag="out_t", bufs=2)
                        nc.vector.tensor_mul(ot, x2, g2_bc)
                        nc.vector.tensor_add(ot, ot, be2_bc)
                        nc.sync.dma_start(
                            out=out.ap()[qt * 128:qt * 128 + 128, :], in_=ot)
            h1_stack.close()
            rtmp_stack.close()

    nc.compile()
    return nc


def _prep_inputs(src, in_proj_w, in_proj_b, out_proj_w, out_proj_b,
                 Qp_w, Qp_b, Kp_w, Kp_b, Vp_w, Vp_b, lam,
                 ff1_w, ff1_b, ff2_w, ff2_b, ln1_g, ln1_b, ln2_g, ln2_b):
    import ml_dtypes
    f = np.float32
    A = lambda x: np.ascontiguousarray(x, dtype=f)
    AB = lambda x: np.ascontiguousarray(np.asarray(x, dtype=f),
                                        dtype=ml_dtypes.bfloat16)
    in_proj_b = np.asarray(in_proj_b, dtype=f)
    in_proj_wT = np.asarray(in_proj_w, dtype=f).T   # [D, 3D]
    cols = np.zeros((NBC, 128), dtype=f)
    cols[CQ:CQ + 8] = in_proj_b[0:D].reshape(8, 128)
    cols[CK:CK + 8] = in_proj_b[D:2 * D].reshape(8, 128)
    cols[CB1:CB1 + 32] = np.asarray(ff1_b, dtype=f).reshape(32, 128)
    cols[CG1:CG1 + 8] = np.asarray(ln1_g, dtype=f).reshape(8, 128)
    cols[CBE1:CBE1 + 8] = np.asarray(ln1_b, dtype=f).reshape(8, 128)
    cols[CQP, 0:R] = np.asarray(Qp_b, dtype=f)
    cols[CKP, 0:R] = np.asarray(Kp_b, dtype=f)
    rows = np.zeros((NRW, D), dtype=f)
    rows[RW_BO] = np.asarray(out_proj_b, dtype=f)
    rows[RW_B12] = np.asarray(ln1_b, dtype=f) + np.asarray(ff2_b, dtype=f)
    rows[RW_G1] = np.asarray(ln1_g, dtype=f)
    rows[RW_G2] = np.asarray(ln2_g, dtype=f)
    rows[RW_BE2] = np.asarray(ln2_b, dtype=f)
    rows[RW_BV] = in_proj_b[2 * D:3 * D]
    rows[RW_BVP] = np.asarray(Vp_b, dtype=f)
    shared = {
        "wqkT": A(in_proj_wT[:, 0:2 * D]),
        "wvT": AB(in_proj_wT[:, 2 * D:3 * D]),
        "woT": AB(np.asarray(out_proj_w).T),
        "vpT": AB(np.asarray(Vp_w).T),
        "qkpT": A(np.concatenate([np.asarray(Qp_w).T, np.asarray(Kp_w).T],
                                 axis=1)),
        "f1T": AB(np.asarray(ff1_w).T),
        "f2T": AB(np.asarray(ff2_w).T),
        "bpack": A(cols.reshape(-1)),
        "rows": A(rows),
        "lam": A(np.asarray(lam)).reshape(1, 1),
    }
    in_maps = []
    for core in range(8):
        b, h = core // 2, core % 2
        srcb = np.asarray(src[b])
        xTb = srcb.T
        if h == 1:
            # own-query columns first (key order is irrelevant to attention)
            xTb = np.concatenate([xTb[:, SQ:], xTb[:, :SQ]], axis=1)
        xTb = np.ascontiguousarray(xTb, dtype=f)
        m = dict(shared)
        m["xT"] = xTb
        m["xb"] = AB(xTb)
        m["x_own"] = A(srcb[h * SQ:(h + 1) * SQ, :])
        in_maps.append(m)
    return in_maps


def _run(inputs, trace=False):
    if "nc" not in _cached:
        _cached["nc"] = _build()
    nc = _cached["nc"]
    in_maps = _prep_inputs(**inputs)
    res = run_bass_kernel_spmd(nc, in_maps, core_ids=list(range(8)),
                               trace=trace)
    out = np.empty((B, S, D), np.float32)
    for core in range(8):
        b, h = core // 2, core % 2
        out[b, h * SQ:(h + 1) * SQ, :] = res.results[core]["out"]
    return out, res


def kernel(**inputs) -> np.ndarray:
    out, _ = _run(inputs, trace=False)
    return out


# revision 18
# speedup vs baseline: 1.2145x; 1.2145x over previous
"""Trainium2 Bass kernel for the EnhancedEncoderLayer (dense MHA + low-rank
top-k sparse attention + FFN, two layernorms).

Sharding: 8 cores = (batch b in 0..3) x (query-half h in {0,1}). Each core
computes output rows [b, h*512:(h+1)*512, :]. K/V-side projections are
computed redundantly per batch pair (no cross-core communication).

The host permutes src[b].T columns so each core's own query tokens are
columns 0..511 (attention contracts over all keys, so key order is
irrelevant); this keeps the SPMD program identical across cores.

v2 structure:
- score-path projections (q, k, sparse Q/K) run in f32r from f32 x / f32
  weights (exp amplifies score errors; bf16 operands there cost ~10x
  output error). V / Vsp / out_proj / FFN operands are bf16.
- V / Vsp are produced token-major directly by matmul (lhsT = x chunks),
  with the bias added via a K=1 ones-row matmul - no PE transposes.
- sparse probabilities computed in BOTH orientations by matmul (psb [q,k]
  for the threshold search, pspT [k,q] for the spmm); the threshold mask
  is applied in place on pspT. psb lives in a never-freed pool so the
  (slow, fully-overlapped) bisection never gates SBUF reuse.
- top-k threshold bisection: 12 iterations of bf16 DVE counting,
  overlapped under the v/vsp/attention phases.
- dense score matmuls issue as adjacent row-group pairs (head A rows
  0:64, head B rows 64:128) sharing one 2-bank PSUM tile and ONE exp.
- engine placement: ACT = exps + evacs (per-partition bias/scale);
  DVE = bisect, k/q evacs, LN, fuse, ctx normalize, masks; GpSimd =
  partition broadcasts + small DMAs ONLY (its ALU is ~6x slower than
  spec); Sync = weight streaming.
"""
import sys
import os
import contextlib

for _p in ('/opt/trn_rl_repo',):
    if _p not in sys.path:
        sys.path.insert(0, _p)

import numpy as np
import concourse.bacc as bacc
import concourse.tile as tile
from concourse import mybir
from concourse.bass_utils import run_bass_kernel_spmd
from concourse.masks import make_identity

F32 = mybir.dt.float32
F32R = mybir.dt.float32r
BF16 = mybir.dt.bfloat16
U32 = mybir.dt.uint32
AF = mybir.ActivationFunctionType
OP = mybir.AluOpType

B, S, D, H, R, DFF = 4, 1024, 1024, 16, 64, 4096
DH = D // H          # 64
SQ = S // 2          # 512 own queries per core
KK = max(1, int(S * 0.2))   # 204
KC = D // 128        # 8 contraction chunks over D
FC = DFF // 128      # 32 chunks over DFF
NQT = SQ // 128      # 4 query tiles
NTOK = S // 128      # 8 token tiles
BISECT_ITERS = 12
INV_SQRT = 0.125     # 1/sqrt(DH) == 1/sqrt(R)

# bpack column layout ([128, NBC] f32)
CQ, CK = 0, 8            # q / k in_proj bias (8 cols each)
CB1 = 24                 # ff1 bias (32 cols)
CG1, CBE1 = 56, 64       # ln1 gamma / beta (8 cols each)
CQP, CKP = 72, 73        # sparse Q/K proj bias (64 rows used)
NBC = 74
# rows layout ([NRW, D] f32)
RW_BO, RW_B12, RW_G1, RW_G2, RW_BE2, RW_BV, RW_BVP = range(7)
NRW = 7

_cached = {}


def _build():
    nc = bacc.Bacc()

    def din(name, shape, dt=F32):
        return nc.declare_dram_parameter(name, list(shape), dt, isOutput=False)

    xT = din("xT", [D, S])              # f32: q/k/sparse projections
    xb = din("xb", [D, S], BF16)        # bf16: v/vsp projections
    x_own = din("x_own", [SQ, D])       # own rows, token-major, f32
    wqkT = din("wqkT", [D, 2 * D])      # f32 (q cols 0:D, k cols D:2D)
    wvT = din("wvT", [D, D], BF16)
    woT = din("woT", [D, D], BF16)
    vpT = din("vpT", [D, D], BF16)
    qkpT = din("qkpT", [D, 2 * R])      # f32
    f1T = din("f1T", [D, DFF], BF16)
    f2T = din("f2T", [DFF, D], BF16)
    bpack = din("bpack", [128 * NBC])
    rows = din("rows", [NRW, D])
    lam = din("lam", [1, 1])
    out = nc.declare_dram_parameter("out", [SQ, D], F32, isOutput=True)
    DBG = bool(os.environ.get("BASSK_DEBUG"))
    if DBG:
        dbg_dense = nc.declare_dram_parameter("dbg_dense", [SQ, D], F32,
                                              isOutput=True)
        dbg_sparse = nc.declare_dram_parameter("dbg_sparse", [SQ, D], F32,
                                               isOutput=True)
        dbg_misc = nc.declare_dram_parameter("dbg_misc", [128, 16], F32,
                                             isOutput=True)

    xT_r = xT.ap().bitcast(F32R).rearrange("(kc p) s -> p kc s", p=128)
    xb_r = xb.ap().rearrange("(kc p) s -> p kc s", p=128)
    wqkT_r = wqkT.ap().bitcast(F32R).rearrange("(kc p) f -> p kc f", p=128)
    wvT_r = wvT.ap().rearrange("(kc p) f -> p kc f", p=128)
    woT_r = woT.ap().rearrange("(kc p) f -> p kc f", p=128)
    vpT_r = vpT.ap().rearrange("(kc p) f -> p kc f", p=128)
    qkpT_r = qkpT.ap().bitcast(F32R).rearrange("(kc p) f -> p kc f", p=128)
    f1T_r = f1T.ap().rearrange("(kc p) f -> p kc f", p=128)
    f2T_r = f2T.ap().rearrange("(kc p) f -> p kc f", p=128)
    bpack_r = bpack.ap().rearrange("(c p) -> p c", p=128)

    with tile.TileContext(nc) as tc:
        est = contextlib.ExitStack()
        with est:
            # ---------------- constants + small loads ----------------
            consts = est.enter_context(tc.tile_pool(name="consts", bufs=1))

            ident_f = consts.tile([128, 128], F32, name="ident_f")
            make_identity(nc, ident_f)

            eps_t = consts.tile([128, 1], F32, name="eps_t")
            nc.vector.memset(eps_t, 1e-5)
            ones1 = consts.tile([128, 1], F32, name="ones1")
            nc.vector.memset(ones1, 1.0)
            ones_bf = consts.tile([1, 128], BF16, name="ones_bf")
            nc.vector.memset(ones_bf, 1.0)
            ones_b1 = consts.tile([128, 1], BF16, name="ones_b1")
            nc.vector.memset(ones_b1, 1.0)

            lam_t = consts.tile([1, 1], F32, name="lam_t")
            nc.gpsimd.dma_start(out=lam_t, in_=lam.ap())
            bp = consts.tile([128, NBC], F32, name="bp")
            nc.gpsimd.dma_start(out=bp, in_=bpack_r)
            qkp_t = consts.tile([128, KC, 2 * R], F32R, name="qkp_t")
            nc.gpsimd.dma_start(out=qkp_t, in_=qkpT_r)
            rtmp_stack = contextlib.ExitStack()
            rtmp = rtmp_stack.enter_context(
                tc.tile_pool(name="rtmp", bufs=2, side="right"))

            def load_row(i):
                rt = rtmp.tile([1, D], F32, name="rrow", tag="rrow")
                nc.gpsimd.dma_start(out=rt, in_=rows.ap()[i:i + 1, :])
                return rt

            sg_t = consts.tile([1, 1], F32, name="sg_t")
            nc.scalar.activation(out=sg_t, in_=lam_t, func=AF.Sigmoid)
            sig_bc = consts.tile([128, 1], F32, name="sig_bc")
            nc.gpsimd.partition_broadcast(sig_bc, sg_t)
            oms_bc = consts.tile([128, 1], F32, name="oms_bc")
            nc.vector.tensor_sub(oms_bc, ones1, sig_bc)

            # bf16 bias rows for the K=1 bias matmuls (v / vsp)
            bvb0 = consts.tile([1, D], BF16, name="bvb0")
            nc.vector.tensor_copy(out=bvb0, in_=load_row(RW_BV))
            bvb1 = consts.tile([1, D], BF16, name="bvb1")
            nc.vector.tensor_copy(out=bvb1, in_=load_row(RW_BVP))

            # own-token residual xot = x_own + sig*bo
            bo_sig = consts.tile([128, D], F32, name="bo_sig")
            nc.gpsimd.partition_broadcast(bo_sig, load_row(RW_BO))
            nc.vector.tensor_scalar_mul(bo_sig, bo_sig, sig_bc)

            xot_pool = est.enter_context(tc.tile_pool(name="xot_pool",
                                                      bufs=1))
            xot = xot_pool.tile([128, NQT, D], F32, name="xot")
            for qt in range(NQT):
                nc.gpsimd.dma_start(
                    out=xot[:, qt, :],
                    in_=x_own.ap()[qt * 128:qt * 128 + 128, :])
            for qt in range(NQT):
                nc.vector.tensor_add(xot[:, qt, :], xot[:, qt, :], bo_sig)

            # x chunks: f32r for the score path, bf16 for v/vsp
            x_pool = est.enter_context(tc.tile_pool(name="x_pool", bufs=1))
            xTt = x_pool.tile([128, KC, S], F32R, name="xTt")
            for kc in range(KC):
                nc.sync.dma_start(out=xTt[:, kc, :], in_=xT_r[:, kc, :])
            xbp_stack = contextlib.ExitStack()
            xbp_pool = xbp_stack.enter_context(
                tc.tile_pool(name="xbp_pool", bufs=1, side="right"))
            xb_t = xbp_pool.tile([128, KC, S], BF16, name="xb_t")
            for kc in range(KC):
                nc.sync.dma_start(out=xb_t[:, kc, :], in_=xb_r[:, kc, :])

            # ---------- long-lived small tiles (incl. bisect) ----------
            bis = est.enter_context(tc.tile_pool(name="bis", bufs=1))
            lo = bis.tile([128, NQT], F32, name="lo")
            hi = bis.tile([128, NQT], F32, name="hi")
            mid = bis.tile([128, NQT], F32, name="mid")
            cnts = bis.tile([128, NQT], F32, name="cnts")
            pred = bis.tile([128, NQT], U32, name="pred")
            rs_sp = bis.tile([128, NQT], F32, name="rs_sp")
            rcp_sp = bis.tile([128, NQT], F32, name="rcp_sp")
            loT_row = bis.tile([1, SQ], F32, name="loT_row")
            thrT = bis.tile([128, SQ], F32, name="thrT")
            psb = bis.tile([128, NQT, S], BF16, name="psb")
            scrb = bis.tile([128, S], BF16, name="scrb")

            ctx_stack = contextlib.ExitStack()
            ctx_pool = ctx_stack.enter_context(
                tc.tile_pool(name="ctx_pool", bufs=1))
            ctxT = ctx_pool.tile([128, KC, SQ], BF16, name="ctxT")

            spv_stack = contextlib.ExitStack()
            spv_pool = spv_stack.enter_context(
                tc.tile_pool(name="spv_pool", bufs=1))
            pspT = spv_pool.tile([128, NTOK, SQ], BF16, name="pspT")
            Vsp = spv_pool.tile([128, NTOK, D], BF16, name="Vsp")

            # ============ phase 0: sparse projections + scores ============
            sp_stack = contextlib.ExitStack()
            sp_pool = sp_stack.enter_context(
                tc.tile_pool(name="sp_pool", bufs=1))
            kspT_b = sp_pool.tile([64, S], BF16, name="kspT_b")
            qspT_b = sp_pool.tile([64, SQ], BF16, name="qspT_b")

            with contextlib.ExitStack() as ph0:
                ps_sp = ph0.enter_context(
                    tc.tile_pool(name="ps_sp", bufs=2, space="PSUM"))
                with nc.named_scope("p0_sparse"):
                    for nh in range(2):
                        ps = ps_sp.tile([64, 512], F32, name="ps0", tag="ps0")
                        for kc in range(KC):
                            nc.tensor.matmul(
                                ps, qkp_t[:, kc, R:2 * R],
                                xTt[:, kc, nh * 512:nh * 512 + 512],
                                start=(kc == 0), stop=(kc == KC - 1))
                        nc.scalar.activation(
                            out=kspT_b[:, nh * 512:nh * 512 + 512], in_=ps,
                            func=AF.Identity, bias=bp[0:64, CKP:CKP + 1],
                            scale=1.0)
                    ps = ps_sp.tile([64, 512], F32, name="ps0", tag="ps0")
                    for kc in range(KC):
                        nc.tensor.matmul(ps, qkp_t[:, kc, 0:R],
                                         xTt[:, kc, 0:SQ],
                                         start=(kc == 0), stop=(kc == KC - 1))
                    nc.scalar.activation(out=qspT_b, in_=ps,
                                         func=AF.Identity,
                                         bias=bp[0:64, CQP:CQP + 1],
                                         scale=1.0)

                with nc.named_scope("p1_ssp"):
                    # psb [q, k] for the threshold search (one exp per qt)
                    for qt in range(NQT):
                        ps = ps_sp.tile([128, 1024], F32, name="ps1",
                                        tag="ps1")
                        for nh in range(2):
                            nc.tensor.matmul(
                                ps[:, nh * 512:nh * 512 + 512],
                                qspT_b[:, qt * 128:qt * 128 + 128],
                                kspT_b[:, nh * 512:nh * 512 + 512],
                                start=True, stop=True)
                        nc.scalar.activation(
                            out=psb[:, qt, :], in_=ps, func=AF.Exp,
                            scale=INV_SQRT)
                    # pspT [k, q] for the spmm (one exp per kt pair)
                    for kt2 in range(NTOK // 2):
                        ps = ps_sp.tile([128, 1024], F32, name="ps1",
                                        tag="ps1")
                        for j in range(2):
                            kt = kt2 * 2 + j
                            nc.tensor.matmul(
                                ps[:, j * 512:j * 512 + 512],
                                kspT_b[:, kt * 128:kt * 128 + 128],
                                qspT_b, start=True, stop=True)
                        nc.scalar.activation(
                            out=pspT[:, kt2 * 2:kt2 * 2 + 2, :], in_=ps,
                            func=AF.Exp, scale=INV_SQRT)

                # ---- top-k threshold bisection (DVE, bf16 counting) ----
                # ~6.5us/iter; fully overlapped under v/vsp/attention
                with nc.named_scope("p2_bisect"):
                    nc.vector.memset(lo, 0.0)
                    nc.vector.memset(hi, 16.0)
                    for it in range(BISECT_ITERS):
                        nc.vector.tensor_tensor(mid, lo, hi, OP.add)
                        nc.vector.tensor_scalar_mul(mid, mid, 0.5)
                        for qt in range(NQT):
                            nc.vector.scalar_tensor_tensor(
                                out=scrb, in0=psb[:, qt, :],
                                scalar=mid[:, qt:qt + 1],
                                in1=ones_b1.to_broadcast([128, S]),
                                op0=OP.is_ge, op1=OP.mult,
                                accum_out=cnts[:, qt:qt + 1])
                        nc.vector.tensor_scalar(out=pred, in0=cnts,
                                                scalar1=float(KK),
                                                scalar2=None, op0=OP.is_ge)
                        nc.vector.copy_predicated(lo, pred, mid)
                        nc.vector.tensor_scalar(out=pred, in0=cnts,
                                                scalar1=float(KK),
                                                scalar2=None, op0=OP.is_lt)
                        nc.vector.copy_predicated(hi, pred, mid)
                    # row-sum of kept values (masked in place, [q,k] side)
                    for qt in range(NQT):
                        nc.vector.scalar_tensor_tensor(
                            out=psb[:, qt, :], in0=psb[:, qt, :],
                            scalar=lo[:, qt:qt + 1], in1=psb[:, qt, :],
                            op0=OP.is_ge, op1=OP.mult,
                            accum_out=rs_sp[:, qt:qt + 1])
                    nc.vector.tensor_scalar(out=rs_sp, in0=rs_sp,
                                            scalar1=1e-9, scalar2=None,
                                            op0=OP.add)
                    nc.vector.reciprocal(rcp_sp, rs_sp)
                    nc.vector.tensor_scalar_mul(rcp_sp, rcp_sp, oms_bc)
                    if DBG:
                        nc.sync.dma_start(out=dbg_misc.ap()[:, 0:4], in_=lo)
                        nc.sync.dma_start(out=dbg_misc.ap()[:, 4:8],
                                          in_=rs_sp)
                        nc.sync.dma_start(out=dbg_misc.ap()[:, 8:12],
                                          in_=rcp_sp)
                        nc.sync.dma_start(out=dbg_misc.ap()[:, 12:16],
                                          in_=cnts)
            sp_stack.close()

            dn_stack = contextlib.ExitStack()
            dn_pool = dn_stack.enter_context(
                tc.tile_pool(name="dn_pool", bufs=1))
            kT = dn_pool.tile([128, KC, S], BF16, name="kT")
            qT = dn_pool.tile([128, KC, SQ], BF16, name="qT")
            Vaug = dn_pool.tile([128, NTOK, H * (DH + 1)], BF16, name="Vaug")
            Vaug_h = Vaug.rearrange("p t (h c) -> p t h c", c=DH + 1)
            nc.gpsimd.memset(Vaug_h[:, :, :, DH:DH + 1], 1.0)

            # ============ phase 3/4: V and Vsp, token-major (bf16) ========
            with contextlib.ExitStack() as ph34:
                wv_pool = ph34.enter_context(
                    tc.tile_pool(name="wv_pool", bufs=3))
                ps_v = ph34.enter_context(
                    tc.tile_pool(name="ps_v", bufs=2, space="PSUM"))

                def vproj(w_r, brow, dest, scope):
                    with nc.named_scope(scope):
                        for nh in range(2):
                            wv = wv_pool.tile([128, KC, 512], BF16,
                                              name="wv", tag="wv")
                            nc.sync.dma_start(
                                out=wv,
                                in_=w_r[:, :, nh * 512:nh * 512 + 512])
                            for t in range(NTOK):
                                ps = ps_v.tile([128, 512], F32, name="psv",
                                               tag="psv")
                                for kc in range(KC):
                                    nc.tensor.matmul(
                                        ps, xb_t[:, kc, t * 128:t * 128 + 128],
                                        wv[:, kc, :],
                                        start=(kc == 0), stop=False)
                                nc.tensor.matmul(
                                    ps, ones_bf,
                                    brow[:, nh * 512:nh * 512 + 512],
                                    start=False, stop=True)
                                if dest is None:
                                    nc.scalar.activation(
                                        out=Vaug_h[:, t, nh * 8:nh * 8 + 8,
                                                   0:DH],
                                        in_=ps, func=AF.Copy, scale=1.0)
                                else:
                                    nc.scalar.activation(
                                        out=dest[:, t,
                                                 nh * 512:nh * 512 + 512],
                                        in_=ps, func=AF.Copy, scale=1.0)

                vproj(wvT_r, bvb0, None, "p3_v")
                vproj(vpT_r, bvb1, Vsp, "p4_vsp")
            xbp_stack.close()

            # ============ phase 5: k/q projections + dense attention ======
            with contextlib.ExitStack() as ph5:
                wstr = ph5.enter_context(tc.tile_pool(name="wstr", bufs=2))
                pt_pool = ph5.enter_context(
                    tc.tile_pool(name="pt_pool", bufs=6))
                rc_pool = ph5.enter_context(
                    tc.tile_pool(name="rc_pool", bufs=2))
                ps_proj = ph5.enter_context(
                    tc.tile_pool(name="ps_proj", bufs=2, space="PSUM"))
                ps_attn = ph5.enter_context(
                    tc.tile_pool(name="ps_attn", bufs=2, space="PSUM"))
                ps_ctx = ph5.enter_context(
                    tc.tile_pool(name="ps_ctx", bufs=2, space="PSUM"))

                with nc.named_scope("p5_kq_attn"):
                    for jj in range(4):
                        for fi in range(2):
                            ft = jj * 2 + fi
                            wkq = wstr.tile([128, KC, 256], F32R, name="wkq",
                                            tag="wkq")
                            nc.sync.dma_start(
                                out=wkq[:, :, 0:128],
                                in_=wqkT_r[:, :, D + ft * 128:
                                           D + ft * 128 + 128])
                            nc.sync.dma_start(
                                out=wkq[:, :, 128:256],
                                in_=wqkT_r[:, :, ft * 128:ft * 128 + 128])
                            for nh in range(2):
                                ps = ps_proj.tile([128, 512], F32, name="psp",
                                                  tag="psp")
                                for kc in range(KC):
                                    nc.tensor.matmul(
                                        ps, wkq[:, kc, 0:128],
                                        xTt[:, kc, nh * 512:nh * 512 + 512],
                                        start=(kc == 0), stop=(kc == KC - 1))
                                nc.vector.tensor_scalar(
                                    out=kT[:, ft, nh * 512:nh * 512 + 512],
                                    in0=ps, scalar1=bp[:, CK + ft:CK + ft + 1],
                                    scalar2=None, op0=OP.add)
                            ps = ps_proj.tile([128, 512], F32, name="psp",
                                              tag="psp")
                            for kc in range(KC):
                                nc.tensor.matmul(
                                    ps, wkq[:, kc, 128:256],
                                    xTt[:, kc, 0:SQ],
                                    start=(kc == 0), stop=(kc == KC - 1))
                            nc.vector.tensor_scalar(
                                out=qT[:, ft, :], in0=ps,
                                scalar1=bp[:, CQ + ft:CQ + ft + 1],
                                scalar2=None, op0=OP.add)
                        # attention for the 4 heads of these two f-tiles
                        for fi in range(2):
                            ft = jj * 2 + fi
                            pts = []
                            for t in range(NTOK):
                                ps = ps_attn.tile([128, 1024], F32,
                                                  name="ps_s", tag="ps_s")
                                for po in (0, 64):
                                    nc.tensor.matmul(
                                        ps[:, (po // 64) * 512:
                                           (po // 64) * 512 + 512],
                                        kT[po:po + 64, ft,
                                           t * 128:t * 128 + 128],
                                        qT[po:po + 64, ft, :],
                                        start=True, stop=True)
                                pt = pt_pool.tile([128, 1024], BF16,
                                                  name="pT", tag="pT")
                                nc.scalar.activation(out=pt, in_=ps,
                                                     func=AF.Exp,
                                                     scale=INV_SQRT)
                                pts.append(pt)
                            for po in (0, 64):
                                hh = 2 * ft + (po // 64)
                                qoff = (po // 64) * 512
                                pctx = ps_ctx.tile([128, 512], F32,
                                                   name="ps_c", tag="ps_c")
                                for t in range(NTOK):
                                    nc.tensor.matmul(
                                        pctx[0:65, :],
                                        Vaug[:, t, hh * 65:hh * 65 + 65],
                                        pts[t][:, qoff:qoff + 512],
                                        start=(t == 0),
                                        stop=(t == NTOK - 1))
                                rsr = rc_pool.tile([1, 512], F32, name="rsr",
                                                   tag="rsr")
                                nc.vector.tensor_copy(out=rsr,
                                                      in_=pctx[64:65, :])
                                rch = rc_pool.tile([1, 512], F32, name="rch",
                                                   tag="rch")
                                nc.vector.reciprocal_approx_fast(out=rch,
                                                                 in_=rsr)
                                rb = rc_pool.tile([64, 512], F32, name="rb",
                                                  tag="rb")
                                nc.gpsimd.partition_broadcast(rb, rch)
                                nc.vector.tensor_mul(
                                    out=ctxT[po:po + 64, ft, :],
                                    in0=pctx[0:64, :], in1=rb)

            dn_stack.close()   # free kT, qT, Vaug

            ds_stack = contextlib.ExitStack()
            ds_pool = ds_stack.enter_context(
                tc.tile_pool(name="ds_pool", bufs=1, side="right"))
            dense_s = ds_pool.tile([128, NQT, D], F32, name="dense_s")
            sparse_s = ds_pool.tile([128, NQT, D], F32, name="sparse_s")

            # ==== threshold row -> mask (overlaps out_proj below) ====
            with contextlib.ExitStack() as ph6a:
                ps_lo = ph6a.enter_context(
                    tc.tile_pool(name="ps_lo", bufs=2, space="PSUM"))
                with nc.named_scope("p6_loT"):
                    for qt in range(NQT):
                        pst = ps_lo.tile([1, 128], F32, name="pslo",
                                         tag="pslo")
                        nc.tensor.transpose(pst, lo[:, qt:qt + 1], ident_f)
                        nc.vector.tensor_copy(
                            out=loT_row[0:1, qt * 128:qt * 128 + 128],
                            in_=pst)
            with nc.named_scope("p6_mask"):
                nc.gpsimd.partition_broadcast(thrT, loT_row)
                for kt in range(NTOK):
                    nc.vector.tensor_tensor(scrb[:, 0:512], pspT[:, kt, :],
                                            thrT, OP.is_ge)
                    nc.vector.tensor_tensor(pspT[:, kt, :], pspT[:, kt, :],
                                            scrb[:, 0:512], OP.mult)

            # ============ phase 7: out_proj, then spmm ============
            with contextlib.ExitStack() as ph7:
                w2str = ph7.enter_context(tc.tile_pool(name="w2str", bufs=1))
                ps_mm = ph7.enter_context(
                    tc.tile_pool(name="ps_mm", bufs=2, space="PSUM"))
                with nc.named_scope("p7_outproj"):
                    wo2 = w2str.tile([128, KC, D], BF16, name="wo2",
                                     tag="wo2")
                    nc.sync.dma_start(out=wo2, in_=woT_r)
                    for qt in range(NQT):
                        ps = ps_mm.tile([128, 1024], F32, name="ps_o",
                                        tag="ps_o")
                        for kc in range(KC):
                            for nh in range(2):
                                nc.tensor.matmul(
                                    ps[:, nh * 512:nh * 512 + 512],
                                    ctxT[:, kc, qt * 128:qt * 128 + 128],
                                    wo2[:, kc, nh * 512:nh * 512 + 512],
                                    start=(kc == 0), stop=(kc == KC - 1))
                        nc.scalar.activation(
                            out=dense_s[:, qt, :], in_=ps, func=AF.Copy,
                            scale=sig_bc)
                with nc.named_scope("p8_spmm"):
                    for qt in range(NQT):
                        ps = ps_mm.tile([128, 1024], F32, name="ps_o",
                                        tag="ps_o")
                        for kt in range(NTOK):
                            for nh in range(2):
                                nc.tensor.matmul(
                                    ps[:, nh * 512:nh * 512 + 512],
                                    pspT[:, kt, qt * 128:qt * 128 + 128],
                                    Vsp[:, kt, nh * 512:nh * 512 + 512],
                                    start=(kt == 0), stop=(kt == NTOK - 1))
                        nc.scalar.activation(
                            out=sparse_s[:, qt, :], in_=ps, func=AF.Copy,
                            scale=rcp_sp[:, qt:qt + 1])
            if DBG:
                for qt in range(NQT):
                    nc.sync.dma_start(
                        out=dbg_dense.ap()[qt * 128:qt * 128 + 128, :],
                        in_=dense_s[:, qt, :])
                    nc.sync.dma_start(
                        out=dbg_sparse.ap()[qt * 128:qt * 128 + 128, :],
                        in_=sparse_s[:, qt, :])
            spv_stack.close()
            ctx_stack.close()

            # ============ phase 9: fuse + LN1 ============
            fse = est.enter_context(tc.tile_pool(name="fse", bufs=1))
            x78_stack = contextlib.ExitStack()
            x78 = x78_stack.enter_context(tc.tile_pool(name="x78", bufs=1))

            g1_bc = fse.tile([128, D], F32, name="g1_bc")
            b12_bc = fse.tile([128, D], F32, name="b12_bc")
            g2_bc = fse.tile([128, D], F32, name="g2_bc")
            be2_bc = fse.tile([128, D], F32, name="be2_bc")
            nc.gpsimd.partition_broadcast(g1_bc, load_row(RW_G1))
            nc.gpsimd.partition_broadcast(b12_bc, load_row(RW_B12))
            nc.gpsimd.partition_broadcast(g2_bc, load_row(RW_G2))
            nc.gpsimd.partition_broadcast(be2_bc, load_row(RW_BE2))

            xhat = x78.tile([128, NQT, D], F32, name="xhat")
            xg = fse.tile([128, NQT, D], F32, name="xg")
            stats = fse.tile([128, NQT, 2, 6], F32, name="stats")
            mv2 = fse.tile([128, NQT, 2], F32, name="mv2")
            sd = fse.tile([128, NQT], F32, name="sd")
            rstd = fse.tile([128, NQT], F32, name="rstd")

            def ln_normalize(x1, qt):
                for half in range(2):
                    nc.vector.bn_stats(
                        out=stats[:, qt, half, :],
                        in_=x1[:, half * 512:half * 512 + 512])
                nc.vector.bn_aggr(out=mv2[:, qt, :], in_=stats[:, qt])
                nc.scalar.activation(out=sd[:, qt:qt + 1],
                                     in_=mv2[:, qt, 1:2], func=AF.Sqrt,
                                     bias=eps_t, scale=1.0)
                nc.vector.reciprocal(rstd[:, qt:qt + 1], sd[:, qt:qt + 1])
                nc.vector.tensor_scalar(out=x1, in0=x1,
                                        scalar1=mv2[:, qt, 0:1],
                                        scalar2=rstd[:, qt:qt + 1],
                                        op0=OP.subtract, op1=OP.mult)

            with nc.named_scope("p9_fuse_ln1"):
                for qt in range(NQT):
                    x1 = xhat[:, qt, :]
                    nc.vector.tensor_add(x1, dense_s[:, qt, :],
                                         sparse_s[:, qt, :])
                    nc.vector.tensor_add(x1, x1, xot[:, qt, :])
                    ln_normalize(x1, qt)
                for qt in range(NQT):
                    nc.vector.tensor_mul(xg[:, qt, :], xhat[:, qt, :],
                                         g1_bc)
                    nc.vector.tensor_add(xg[:, qt, :], xg[:, qt, :],
                                         b12_bc)
            ds_stack.close()

            # ============ phase 10: xhat transpose -> ff1 input ============
            xln_stack = contextlib.ExitStack()
            xln_pool = xln_stack.enter_context(
                tc.tile_pool(name="xln_pool", bufs=1, side="right"))
            xlnT = xln_pool.tile([128, KC, SQ], BF16, name="xlnT")
            with contextlib.ExitStack() as ph10:
                ps_tr3 = ph10.enter_context(
                    tc.tile_pool(name="ps_tr3", bufs=2, space="PSUM"))
                with nc.named_scope("p10_xT"):
                    for qt in range(NQT):
                        for fc in range(KC):
                            pst = ps_tr3.tile([128, 128], F32, name="pst3",
                                              tag="pst3")
                            nc.tensor.transpose(
                                pst, xhat[:, qt, fc * 128:fc * 128 + 128],
                                ident_f)
                            nc.scalar.activation(
                                out=xlnT[:, fc, qt * 128:qt * 128 + 128],
                                in_=pst, func=AF.Identity,
                                bias=bp[:, CBE1 + fc:CBE1 + fc + 1],
                                scale=bp[:, CG1 + fc:CG1 + fc + 1])
            x78_stack.close()

            # ============ phase 11: ff1 + relu ============
            h1_stack = contextlib.ExitStack()
            h1_pool = h1_stack.enter_context(
                tc.tile_pool(name="h1_pool", bufs=1))
            h1T = h1_pool.tile([128, FC, SQ], BF16, name="h1T")
            with contextlib.ExitStack() as ph11:
                w3str = ph11.enter_context(tc.tile_pool(name="w3str", bufs=2))
                ps_f1 = ph11.enter_context(
                    tc.tile_pool(name="ps_f1", bufs=4, space="PSUM"))
                with nc.named_scope("p11_ff1"):
                    for jj in range(16):
                        wt = w3str.tile([128, KC, 256], BF16, name="w1t",
                                        tag="w3")
                        f0 = jj * 256
                        nc.sync.dma_start(out=wt,
                                          in_=f1T_r[:, :, f0:f0 + 256])
                        for fi in range(2):
                            dft = jj * 2 + fi
                            ps = ps_f1.tile([128, 512], F32, name="ps_f",
                                            tag="ps_f")
                            for kc in range(KC):
                                nc.tensor.matmul(
                                    ps, wt[:, kc, fi * 128:fi * 128 + 128],
                                    xlnT[:, kc, :],
                                    start=(kc == 0), stop=(kc == KC - 1))
                            nc.scalar.activation(
                                out=h1T[:, dft, :], in_=ps, func=AF.Relu,
                                bias=bp[:, CB1 + dft:CB1 + dft + 1],
                                scale=1.0)
            xln_stack.close()

            # ============ phase 12: ff2 + residual + LN2 + out ============
            ff_s = fse.tile([128, NQT, D], F32, name="ff_s")
            with contextlib.ExitStack() as ph12:
                w4str = ph12.enter_context(tc.tile_pool(name="w4str", bufs=2))
                ps_f2 = ph12.enter_context(
                    tc.tile_pool(name="ps_f2", bufs=4, space="PSUM"))
                with nc.named_scope("p12_ff2"):
                    pss = [ps_f2.tile([128, 1024], F32, name="ps_g",
                                      tag="ps_g") for _ in range(NQT)]
                    for k4 in range(FC // 4):
                        f2h = w4str.tile([128, 4, D], BF16, name="f2h",
                                         tag="w4")
                        nc.sync.dma_start(out=f2h,
                                          in_=f2T_r[:, k4 * 4:k4 * 4 + 4, :])
                        for j in range(4):
                            kc = k4 * 4 + j
                            for qt in range(NQT):
                                for nh in range(2):
                                    nc.tensor.matmul(
                                        pss[qt][:, nh * 512:nh * 512 + 512],
                                        h1T[:, kc, qt * 128:qt * 128 + 128],
                                        f2h[:, j, nh * 512:nh * 512 + 512],
                                        start=(kc == 0),
                                        stop=(kc == FC - 1))
                    for qt in range(NQT):
                        nc.scalar.activation(
                            out=ff_s[:, qt, :], in_=pss[qt], func=AF.Copy,
                            scale=1.0)

                with nc.named_scope("p12_ln2"):
                    for qt in range(NQT):
                        x2 = ff_s[:, qt, :]
                        nc.vector.tensor_add(x2, x2, xg[:, qt, :])
                        ln_normalize(x2, qt)
                        ot = fse.tile([128, D], F32, name="out_t",
                                      tag="out_t", bufs=2)
                        nc.vector.tensor_mul(ot, x2, g2_bc)
                        nc.vector.tensor_add(ot, ot, be2_bc)
                        nc.sync.dma_start(
                            out=out.ap()[qt * 128:qt * 128 + 128, :], in_=ot)
            h1_stack.close()
            rtmp_stack.close()

    nc.compile()
    return nc


def _prep_inputs(src, in_proj_w, in_proj_b, out_proj_w, out_proj_b,
                 Qp_w, Qp_b, Kp_w, Kp_b, Vp_w, Vp_b, lam,
                 ff1_w, ff1_b, ff2_w, ff2_b, ln1_g, ln1_b, ln2_g, ln2_b):
    import ml_dtypes
    f = np.float32
    A = lambda x: np.ascontiguousarray(x, dtype=f)
    AB = lambda x: np.ascontiguousarray(np.asarray(x, dtype=f),
                                        dtype=ml_dtypes.bfloat16)
    in_proj_b = np.asarray(in_proj_b, dtype=f)
    in_proj_wT = np.asarray(in_proj_w, dtype=f).T   # [D, 3D]
    cols = np.zeros((NBC, 128), dtype=f)
    cols[CQ:CQ + 8] = in_proj_b[0:D].reshape(8, 128)
    cols[CK:CK + 8] = in_proj_b[D:2 * D].reshape(8, 128)
    cols[CB1:CB1 + 32] = np.asarray(ff1_b, dtype=f).reshape(32, 128)
    cols[CG1:CG1 + 8] = np.asarray(ln1_g, dtype=f).reshape(8, 128)
    cols[CBE1:CBE1 + 8] = np.asarray(ln1_b, dtype=f).reshape(8, 128)
    cols[CQP, 0:R] = np.asarray(Qp_b, dtype=f)
    cols[CKP, 0:R] = np.asarray(Kp_b, dtype=f)
    rows = np.zeros((NRW, D), dtype=f)
    rows[RW_BO] = np.asarray(out_proj_b, dtype=f)
    rows[RW_B12] = np.asarray(ln1_b, dtype=f) + np.asarray(ff2_b, dtype=f)
    rows[RW_G1] = np.asarray(ln1_g, dtype=f)
    rows[RW_G2] = np.asarray(ln2_g, dtype=f)
    rows[RW_BE2] = np.asarray(ln2_b, dtype=f)
    rows[RW_BV] = in_proj_b[2 * D:3 * D]
    rows[RW_BVP] = np.asarray(Vp_b, dtype=f)
    shared = {
        "wqkT": A(in_proj_wT[:, 0:2 * D]),
        "wvT": AB(in_proj_wT[:, 2 * D:3 * D]),
        "woT": AB(np.asarray(out_proj_w).T),
        "vpT": AB(np.asarray(Vp_w).T),
        "qkpT": A(np.concatenate([np.asarray(Qp_w).T, np.asarray(Kp_w).T],
                                 axis=1)),
        "f1T": AB(np.asarray(ff1_w).T),
        "f2T": AB(np.asarray(ff2_w).T),
        "bpack": A(cols.reshape(-1)),
        "rows": A(rows),
        "lam": A(np.asarray(lam)).reshape(1, 1),
    }
    in_maps = []
    for core in range(8):
        b, h = core // 2, core % 2
        srcb = np.asarray(src[b])
        xTb = srcb.T
        if h == 1:
            # own-query columns first (key order is irrelevant to attention)
            xTb = np.concatenate([xTb[:, SQ:], xTb[:, :SQ]], axis=1)
        xTb = np.ascontiguousarray(xTb, dtype=f)
        m = dict(shared)
        m["xT"] = xTb
        m["xb"] = AB(xTb)
        m["x_own"] = A(srcb[h * SQ:(h + 1) * SQ, :])
        in_maps.append(m)
    return in_maps


def _run(inputs, trace=False):
    if "nc" not in _cached:
        _cached["nc"] = _build()
    nc = _cached["nc"]
    in_maps = _prep_inputs(**inputs)
    res = run_bass_kernel_spmd(nc, in_maps, core_ids=list(range(8)),
                               trace=trace)
    out = np.empty((B, S, D), np.float32)
    for core in range(8):
        b, h = core // 2, core % 2
        out[b, h * SQ:(h + 1) * SQ, :] = res.results[core]["out"]
    return out, res


def kernel(**inputs) -> np.ndarray:
    out, _ = _run(inputs, trace=False)
    return out
